# revision 31
# baseline (speedup 1.0000x reference)
"""Trainium2 Bass kernel for nn_AutoSelectAttention (dynamic-span Gaussian
attention scores with the skew/reshape band-extraction trick).

Math: reference builds y[b,m,j] = -((x[j]+mean)/(var+eps))^2 with
x = arange(-2L, 2L), then skew-reshapes to (B, S, L, 3L).  The reshape
trick collapses to: out[b, s, i, k] = -((k - i - L + mean_m)/(var_m+eps))^2
with m = s*L + i, k in [0, 3L).  So each token emits one 3L-wide quadratic
band; pure data-parallel over batch (1 batch per NeuronCore).

Per-core device pipeline (tokens tiled 128/partition-block, 32 blocks):
  GPS:  iota kgrid (k = 0..3071, in 4 column chunks) and offs (i+L) —
        on-device constants, generated during the span DMA
  DVE:  per-token u = 1/(var+eps), bias = (mean - i - L) * u
  ACT:  sq = Square(kgrid * u[p] + bias[p])
  DVE:  ng = sq * -1
  DMA:  ng -> out rows (1.5 MiB contiguous per block), sync/HWDGE ring

The kernel is HBM-write-bound (~48 MiB/core at ~428 GB/s => ~118 us); the
ramp is minimized by chunking the first blocks and computing the block-0
scalars before the rest.

TRN2 constraint honored throughout: an ACT instruction can carry only ONE
semaphore wait.  Every Square's operands resolve to a single DVE wait: the
u/bias scalars are DVE-produced, sq tiles are only ever consumed by DVE,
and the gpsimd-produced kgrid is "observed" once per chunk by a 1-column
touch Square (whose single wait is the Pool semaphore), after which real
Squares reading kgrid need no additional wait.
"""

import sys

import numpy as np

sys.path.insert(0, "/opt/trn_rl_repo")

import concourse.bass as bass  # noqa: F401  (engine types, ts helpers)
import concourse.tile as tile
from concourse import bacc, mybir
from concourse.bass_utils import run_bass_kernel_spmd

B = 8
M = 4096
L = M // 4          # 1024
S = M // L          # 4
W = 3 * L           # 3072 output band width
P = 128             # partitions
NT = M // P         # 32 token-blocks per core
EPS = 1e-5
NCORES = 8
CH = 4              # column chunks for the first token-block
CW = W // CH        # 768

_PROG = None


def _build_program():
    nc = bacc.Bacc("TRN2", target_bir_lowering=False, debug=False)
    fp32 = mybir.dt.float32

    span_t = nc.dram_tensor("span_t", [P, 2 * NT], fp32, kind="ExternalInput")
    out = nc.dram_tensor("out", [M, W], fp32, kind="ExternalOutput")

    with tile.TileContext(nc) as tc:
        with (
            tc.tile_pool(name="const", bufs=1) as cpool,
            tc.tile_pool(name="sqp", bufs=4) as sqpool,
            tc.tile_pool(name="ngp", bufs=8) as ngpool,
            tc.tile_pool(name="tp", bufs=CH) as tpool,
        ):
            # span load first: everything downstream gates on it.
            sp = cpool.tile([P, 2 * NT], fp32)
            nc.sync.dma_start(sp[:], span_t.ap())

            # On-device constants (gpsimd, runs during the span DMA):
            # off_t[p, t] = 128*(t%8) + p + L  (= i + L); kgi[p, k] = k.
            # offs first (prep gates on it), then kgi in chunks so the
            # first kg copy can start ~1.3us after gpsimd wakes instead
            # of 5.3us (full-iota latency).
            off_t = cpool.tile([P, NT], fp32)
            nc.gpsimd.iota(
                off_t[:],
                [[0, NT // 8], [128, 8]],
                base=L,
                channel_multiplier=1,
                allow_small_or_imprecise_dtypes=True,
            )
            kgi = cpool.tile([P, W], fp32)
            for c in range(CH):
                cs, ce = c * CW, (c + 1) * CW
                nc.gpsimd.iota(
                    kgi[:, cs:ce],
                    [[1, CW]],
                    base=cs,
                    channel_multiplier=0,
                    allow_small_or_imprecise_dtypes=True,
                )

            # Per-token scalars: u = 1/(var+eps), bias = (mean - i - L) * u.
            # Column 0 (token-block 0) first so the first Square can start
            # as soon as the span DMA lands, then the remaining 31 columns.
            dvar = cpool.tile([P, NT], fp32)
            u = cpool.tile([P, NT], fp32)
            cm = cpool.tile([P, NT], fp32)
            bb = cpool.tile([P, NT], fp32)
            nc.vector.tensor_scalar_add(dvar[:, 0:1], sp[:, NT : NT + 1], EPS)
            nc.vector.reciprocal(u[:, 0:1], dvar[:, 0:1])
            nc.vector.tensor_sub(cm[:, 0:1], sp[:, 0:1], off_t[:, 0:1])
            bb0_inst = nc.vector.tensor_mul(bb[:, 0:1], cm[:, 0:1], u[:, 0:1])

            out_ap = out.ap()

            # Token-block 0, in column chunks: store stream starts early.
            # Before the Square of chunk c, a 1-column "touch" Square reads
            # that kgi chunk: the touch carries the single Pool(iota) wait,
            # after which ACT has observed the gpsimd tick and the real
            # Squares read kgi directly with only their DVE wait (TRN2 ACT
            # codegen allows one sync-wait per instruction).  Touches use
            # func=Square so no ACT table switch is triggered.
            sq0 = sqpool.tile([P, W], fp32, tag="sq")
            ng0 = ngpool.tile([P, W], fp32, tag="ng")
            prev_sq_inst = None
            for c in range(CH):
                cs, ce = c * CW, (c + 1) * CW
                touch = tpool.tile([P, 1], fp32, tag="touch")
                t_inst = nc.scalar.activation(
                    touch[:], kgi[:, cs : cs + 1],
                    mybir.ActivationFunctionType.Square,
                )
                if prev_sq_inst is not None:
                    # Order-only edge: keep touches interleaved with the
                    # Squares on ACT instead of scheduler-grouped up front.
                    tile.add_dep_helper(
                        t_inst.ins,
                        prev_sq_inst,
                        sync=False,
                        reason="interleave kgi touches with first-block squares",
                    )
                s_inst = nc.scalar.activation(
                    sq0[:, cs:ce],
                    kgi[:, cs:ce],
                    mybir.ActivationFunctionType.Square,
                    bias=bb[:, 0:1],
                    scale=u[:, 0:1],
                )
                prev_sq_inst = s_inst.ins
                nc.vector.tensor_scalar_mul(ng0[:, cs:ce], sq0[:, cs:ce], -1.0)
                nc.sync.dma_start(out_ap[0:P, cs:ce], ng0[:, cs:ce])

            # Remaining 31 columns of the per-token scalars — emitted after
            # block 0 and order-pinned behind the column-0 chain so the
            # scheduler cannot hoist them ahead of it.
            rest_inst = nc.vector.tensor_scalar_add(
                dvar[:, 1:NT], sp[:, NT + 1 : 2 * NT], EPS
            )
            tile.add_dep_helper(
                rest_inst.ins,
                bb0_inst.ins,
                sync=False,
                reason="column-0 scalars first",
            )
            nc.vector.reciprocal(u[:, 1:NT], dvar[:, 1:NT])
            nc.vector.tensor_sub(cm[:, 1:NT], sp[:, 1:NT], off_t[:, 1:NT])
            nc.vector.tensor_mul(bb[:, 1:NT], cm[:, 1:NT], u[:, 1:NT])

            # Token-blocks 1-4 in halves: keeps the young store stream fed
            # while the full-block pipeline is still filling.
            for t in range(1, 5):
                sq1 = sqpool.tile([P, W], fp32, tag="sq")
                ng1 = ngpool.tile([P, W], fp32, tag="ng")
                for c in range(2):
                    cs, ce = c * (W // 2), (c + 1) * (W // 2)
                    nc.scalar.activation(
                        sq1[:, cs:ce],
                        kgi[:, cs:ce],
                        mybir.ActivationFunctionType.Square,
                        bias=bb[:, t : t + 1],
                        scale=u[:, t : t + 1],
                    )
                    nc.vector.tensor_scalar_mul(ng1[:, cs:ce], sq1[:, cs:ce], -1.0)
                    nc.sync.dma_start(out_ap[t * P : (t + 1) * P, cs:ce], ng1[:, cs:ce])

            for t in range(5, NT):
                sq = sqpool.tile([P, W], fp32, tag="sq")
                nc.scalar.activation(
                    sq[:],
                    kgi[:],
                    mybir.ActivationFunctionType.Square,
                    bias=bb[:, t : t + 1],
                    scale=u[:, t : t + 1],
                )
                ng = ngpool.tile([P, W], fp32, tag="ng")
                nc.vector.tensor_scalar_mul(ng[:], sq[:], -1.0)
                nc.sync.dma_start(out_ap[t * P : (t + 1) * P, :], ng[:])
    nc.compile()
    return nc


def _in_maps(span: np.ndarray):
    maps = []
    for b in range(B):
        mean_t = np.ascontiguousarray(span[b, :, 0].reshape(NT, P).T)
        var_t = np.ascontiguousarray(span[b, :, 1].reshape(NT, P).T)
        span_tb = np.concatenate([mean_t, var_t], axis=1)
        maps.append({"span_t": span_tb})
    return maps


def _get_program():
    global _PROG
    if _PROG is None:
        _PROG = _build_program()
    return _PROG


def run(span: np.ndarray, **spmd_kwargs):
    """Run the SPMD kernel; returns (output array (B,S,L,W), BassKernelResults)."""
    prog = _get_program()
    res = run_bass_kernel_spmd(prog, _in_maps(span), list(range(NCORES)), **spmd_kwargs)
    out = np.stack(
        [res.results[b]["out"].reshape(S, L, W) for b in range(B)], axis=0
    )
    return out, res


def kernel(**inputs: np.ndarray) -> np.ndarray:
    span = np.ascontiguousarray(np.asarray(inputs["span"], dtype=np.float32))
    assert span.shape == (B, M, 2), span.shape
    out, _ = run(span)
    return out
